# revision 96
# baseline (speedup 1.0000x reference)
"""Trainium2 Bass kernel for GATRelationNet (self-contained).

Math:
  att_h = attributes @ att_w                        [N, H]
  e     = leaky_relu(att_h@a1 + (att_h@a2).T, 0.2)  [N, N]
  attn  = softmax(e, axis=1)
  att_outs = attn @ att_h                           [N, H]
  img_proj = image_feats @ img_w                    [B, H]
  sem_proj = att_outs @ sem_w + sem_b               [N, H]
  out[b,n] = fc_b + sum_h fc_w[h]*relu(img_proj[b,h] + sem_proj[n,h])

Strategy (8 cores, GAT replicated, relation sharded over batch):
  - Weight folds on host: w12 = att_w @ [a1|a2]  (logit projections) and
    W2 = att_w @ sem_w, so sem_proj^T (unnormalized) is
    EP[h',i] = sum_j P[j,h'] * exp(e[i,j]) with P = attributes @ W2.
    att_h is never materialized on device.
  - bf16 operands throughout the GAT (f32 PSUM accumulation), which also
    enables the DVE 4x element mode for the big relu pass.
  - e-chain per j-chunk pipelines across engines (ACT fused Prelu for the
    first chunks, DVE z+leaky for the rest, ACT exp); P matmuls, the
    broadcast-form colsum, and EP-m0 partials interleave per chunk on PE
    so it stays continuously busy (full-clock pstate) behind the exps.
  - The colsum matmul uses a [JW,128] all-ones stationary so every psum
    row holds the column sum; the softmax reciprocal runs directly on
    that broadcast (one custom-DVE op), feeding sem2 = EP * (1/s).
  - Relation: relu tiles r[m,b] = relu(sem2T[m] + imgb[m][:,b]) [128,1000]
    produced whole-tile on DVE/ACT/GPSIMD (statically load balanced); the
    fc reduction uses r slices as the matmul *stationary* operand with the
    fc_w chunk moving -> out [125,1] per matmul (cost = 1 output column).
    Four single-m waves each accumulate into their own PSUM tile
    [125, 8j*32b] with one-shot groups (PSUM zero-region-safe); the
    output is the sum of the four, assembled in b-halves to pipeline the
    final copy + DMA.
  - Output leaves the device as out^T [N, BS]; the host transposes and
    adds fc_b.
"""

import numpy as np
import ml_dtypes

import concourse.bass as bass
import concourse.mybir as mybir
import concourse.tile as tile
from concourse import bacc
from concourse.bass_utils import run_bass_kernel_spmd

P = 128
B, N, A, H, IDIM = 256, 1000, 512, 512, 512
NCORES = 8
BS = B // NCORES      # 32 batch rows per core
KA = A // P           # 4 contraction chunks over A (and over H for EP)
HM = H // P           # 4 h chunks
NJ = 8                # j chunks
JW = N // NJ          # 125
IW = 500              # i half width (PSUM bank = 512 fp32)
NEG = 0.2

F32 = mybir.dt.float32
BF = mybir.dt.bfloat16
AF = mybir.ActivationFunctionType
OP = mybir.AluOpType

# relation relu producer split: engine per (m,b) tile index, built from
# rate weights (ns per [128,1000] tile) measured in TimelineSim.
RELU_DVE, RELU_ACT, RELU_GPS = 83, 26, 19   # sums to 128


def _relu_schedule():
    """Engine per relu tile (2 waves x 64 tiles), interleaved so each
    engine's queue drains evenly."""
    shares = {"D": RELU_DVE, "A": RELU_ACT, "G": RELU_GPS}
    acc = {k: 0.0 for k in shares}
    out = []
    for _ in range(128):
        for k in shares:
            acc[k] += shares[k] / 128.0
        pick = max(acc, key=lambda k: acc[k])
        acc[pick] -= 1.0
        out.append(pick)
    return out


_CACHE = {}


def _build_program():
    if "nc" in _CACHE:
        return _CACHE["nc"]

    nc = bacc.Bacc(
        "TRN2", target_bir_lowering=False, debug=False, num_devices=NCORES
    )

    d_attrT = nc.dram_tensor("attrT", [P, KA * N], BF, kind="ExternalInput")
    d_w12 = nc.dram_tensor("w12", [P, KA * 2], BF, kind="ExternalInput")
    d_w1rep = nc.dram_tensor("w1rep", [P, KA * JW], BF, kind="ExternalInput")
    d_w2 = nc.dram_tensor("w2", [P, KA * H], BF, kind="ExternalInput")
    d_img_w = nc.dram_tensor("img_w", [P, KA * H], BF, kind="ExternalInput")
    d_imgfT = nc.dram_tensor("imgfT", [P, KA * BS], BF, kind="ExternalInput")
    d_sem_bT = nc.dram_tensor("sem_bT", [P, HM], F32, kind="ExternalInput")
    d_fcw = nc.dram_tensor("fcw", [P, HM], BF, kind="ExternalInput")
    d_out = nc.dram_tensor("out", [N, BS], F32, kind="ExternalOutput")

    with tile.TileContext(nc) as tc:
        _program(nc, tc, d_attrT, d_w12, d_w1rep, d_w2, d_img_w,
                 d_imgfT, d_sem_bT, d_fcw, d_out)

    nc.compile()
    _CACHE["nc"] = nc
    return nc


def _program(nc, tc, d_attrT, d_w12, d_w1rep, d_w2, d_img_w,
             d_imgfT, d_sem_bT, d_fcw, d_out):
    cpool_ctx = tc.tile_pool(name="consts", bufs=1)
    cpool = cpool_ctx.__enter__()

    # ---- persistent SBUF tensors ----
    attrT = cpool.tile([P, KA * N], BF, tag="attrT", name="attrT")
    attrk = [attrT[:, k * N:(k + 1) * N] for k in range(KA)]
    w12 = cpool.tile([P, KA * 2], BF, tag="w12", name="w12")
    w1rep = cpool.tile([P, KA * JW], BF, tag="w1rep", name="w1rep")
    w2 = cpool.tile([P, KA * H], BF, tag="w2", name="w2")
    w2k = [w2[:, k * H:(k + 1) * H] for k in range(KA)]
    imgw = cpool.tile([P, KA * H], BF, tag="imgw", name="imgw")
    imgfT = cpool.tile([P, KA * BS], BF, tag="imgfT", name="imgfT")
    sembT = cpool.tile([P, HM], F32, tag="sembT", name="sembT")
    fcw = cpool.tile([P, HM], BF, tag="fcw", name="fcw")

    ones_row = cpool.tile([1, P], BF, tag="ones_row", name="ones_row")
    ones_bc = cpool.tile([JW, P], BF, tag="ones_bc", name="ones_bc")
    f1b = cpool.tile([JW, N], BF, tag="f1b", name="f1b")
    f2col = cpool.tile([JW, 2 * NJ], F32, tag="f2col", name="f2col")
    imgb = cpool.tile([P, HM * BS], F32, tag="imgb", name="imgb")
    expT = [cpool.tile([JW, N], BF, tag=f"expT{j}", name=f"expT{j}")
            for j in range(NJ)]
    Psb = [cpool.tile([JW, H], BF, tag=f"P{j}", name=f"P{j}")
           for j in range(NJ)]
    rb = cpool.tile([P, N], F32, tag="rb", name="rb")
    sem2T = [cpool.tile([P, N], BF, tag=f"s2{m}", name=f"s2{m}")
             for m in range(HM)]
    out_sb = cpool.tile([JW, NJ * BS], F32, tag="out_sb", name="out_sb")

    # ---- loads (ordered by first use; attrT chunked so the logit
    # matmuls start as each contraction chunk lands) ----
    nc.sync.dma_start(w1rep[:], d_w1rep[:, :])
    nc.sync.dma_start(w12[:], d_w12[:, :])
    for k in range(KA):
        ksl = slice(k * N, (k + 1) * N)
        nc.sync.dma_start(attrT[:, ksl], d_attrT[:, ksl])
    nc.sync.dma_start(w2[:], d_w2[:, :])
    nc.sync.dma_start(imgw[:], d_img_w[:, :])
    nc.sync.dma_start(imgfT[:], d_imgfT[:, :])
    nc.sync.dma_start(sembT[:], d_sem_bT[:, :])
    nc.sync.dma_start(fcw[:], d_fcw[:, :])

    nc.vector.memset(ones_row[:], 1.0)
    nc.vector.memset(ones_bc[:], 1.0)

    # warm the gpsimd ucode ops early (op load is ~us on real hw)
    gwarm = cpool.tile([P, 8], F32, tag="gwarm", name="gwarm")
    nc.vector.memset(gwarm[:], 0.0)
    nc.gpsimd.tensor_scalar(
        gwarm[:], gwarm[:], 0.0, 0.0, op0=OP.add, op1=OP.max
    )
    nc.gpsimd.tensor_tensor(
        gwarm[:], gwarm[:], gwarm[:], op=OP.mult
    )

    # ---- phase A: warmup, f1, f2, f1b, imgb (small PE matmuls) ----
    with tc.tile_pool(name="psA", bufs=1, space="PSUM") as psA:
        # PE pstate warmup: ~3us of continuous dummy matmuls while the
        # attrT DMA lands, so f1/f2/P run at full clock (0.42ns/col
        # instead of 0.83).
        wsrc = cpool.tile([1, IW], BF, tag="wsrc", name="wsrc")
        nc.vector.memset(wsrc[:], 0.0)
        for w in range(5):
            wps = psA.tile([P, IW], F32, tag="warm", name="warm", bufs=2)
            nc.tensor.matmul(wps[:], ones_row[0:1, :], wsrc[:],
                             start=True, stop=True)

        # f1b [125, N] produced directly by PE via host-replicated w1 columns
        for ih in range(2):
            isl = slice(ih * IW, (ih + 1) * IW)
            fps = psA.tile([JW, IW], F32, tag="f1b", name="f1b", bufs=2)
            for k in range(KA):
                nc.tensor.matmul(
                    fps[:], w1rep[:, k * JW:(k + 1) * JW], attrk[k][:, isl],
                    start=(k == 0), stop=(k == KA - 1),
                )
            nc.vector.tensor_copy(f1b[0:JW, isl], fps[:])

        f2ps = psA.tile([JW, 2 * NJ], F32, tag="f2", name="f2")
        for j in range(NJ):
            jsl = slice(j * JW, (j + 1) * JW)
            for k in range(KA):
                nc.tensor.matmul(
                    f2ps[:, 2 * j:2 * j + 2], attrk[k][:, jsl],
                    w12[:, 2 * k:2 * k + 2],
                    start=(k == 0), stop=(k == KA - 1),
                )
        nc.vector.tensor_copy(f2col[:], f2ps[:])

    # ---- phases B+C interleaved so PE stays continuously busy ----
    # j-loop: e-chain (DVE z -> DVE/GPS leaky -> ACT exp), P[j] matmuls,
    # EP-m0 + colsum partials.  Then EP-m1 / relation-m0 / EP-m2 /
    # relation-m1 / ... alternate: each EP-m block keeps PE busy while the
    # relu producers work, and each relation-m block drains fast.
    sched = _relu_schedule()
    epool_ctx = tc.tile_pool(name="et", bufs=3)
    epool = epool_ctx.__enter__()
    rpool_ctx = tc.tile_pool(name="relu", bufs=40)
    rpool = rpool_ctx.__enter__()

    def relu_tile(m, b, eng):
        r = rpool.tile([P, N], BF, tag="r", name="r")
        bias = imgb[:, m * BS + b:m * BS + b + 1]
        if eng == "A":
            nc.scalar.activation(r[:], sem2T[m][:], AF.Relu, bias=bias)
        elif eng == "D":
            nc.vector.tensor_scalar(
                r[:], sem2T[m][:], bias, 0.0, op0=OP.add, op1=OP.max
            )
        else:
            nc.gpsimd.tensor_scalar(
                r[:], sem2T[m][:], bias, 0.0, op0=OP.add, op1=OP.max
            )
        return r

    def relation_wave(m, p_acc, b0=0, b1=BS):
        # single-m wave: each PSUM column group is one start+stop matmul,
        # so only one accumulation group is ever open per zero region
        rs = {b: relu_tile(m, b, sched[m * BS + b]) for b in range(b0, b1)}
        for b in range(b0, b1):
            for j in range(NJ):
                jsl = slice(j * JW, (j + 1) * JW)
                nc.tensor.matmul(
                    p_acc[:, j * BS + b:j * BS + b + 1],
                    rs[b][:, jsl], fcw[:, m:m + 1],
                    start=True, stop=True,
                )

    def ep_block(pool, m, prefix):
        tiles = [pool.tile([P, IW], F32, tag=f"{prefix}{ih}",
                           name=f"{prefix}{ih}") for ih in range(2)]
        msl = slice(m * P, (m + 1) * P)
        for j in range(NJ):
            for ih in range(2):
                isl = slice(ih * IW, (ih + 1) * IW)
                nc.tensor.matmul(
                    tiles[ih][:], Psb[j][:, msl], expT[j][:, isl],
                    start=(j == 0), stop=(j == NJ - 1),
                )
        return tiles

    def sem2_block(m, tiles):
        # GPSIMD cannot touch PSUM: m0 multiplies on DVE (gate-critical);
        # the rest stage through SBUF via ACT and multiply on GPS
        for ih in range(2):
            isl = slice(ih * IW, (ih + 1) * IW)
            if m < 2:
                nc.vector.tensor_tensor(
                    sem2T[m][:, isl], tiles[ih][:], rb[:, isl], op=OP.mult,
                )
            else:
                scr = epool.tile([P, IW], BF, tag="s2scr", name="s2scr")
                nc.scalar.copy(scr[:], tiles[ih][:])
                nc.gpsimd.tensor_tensor(
                    sem2T[m][:, isl], scr[:], rb[:, isl], op=OP.mult,
                )

    def e_chain(j):
        # first chunks: fused Prelu on ACT (fills ACT while the DVE/GPS
        # leakys for later chunks run ahead); exp always on ACT, which is
        # the pipeline pacer.
        e_t = epool.tile([JW, N], BF, tag="e", name="e")
        if j < 4:
            nc.scalar.activation(
                e_t[:], f1b[:], AF.Prelu,
                bias=f2col[:, 2 * j + 1:2 * j + 2], alpha=NEG,
            )
        else:
            nc.vector.tensor_scalar(
                e_t[:], f1b[:], f2col[:, 2 * j + 1:2 * j + 2],
                None, op0=OP.add,
            )
            nc.vector.scalar_tensor_tensor(
                e_t[:], e_t[:], NEG, e_t[:], op0=OP.mult, op1=OP.max
            )
        nc.scalar.activation(expT[j][:], e_t[:], AF.Exp)

    if True:
        with tc.tile_pool(name="ps1", bufs=1, space="PSUM") as ps1:
            # colsum accumulated directly in broadcast form: lhsT is a full
            # [JW, P] ones block, so every psum row holds the column sum and
            # the reciprocal can run straight on it (no copy/bcast hops)
            csps = [ps1.tile([P, IW], F32, tag=f"cs{ih}", name=f"cs{ih}")
                    for ih in range(2)]
            ep0 = [ps1.tile([P, IW], F32, tag=f"ep0_{ih}",
                            name=f"ep0_{ih}") for ih in range(2)]

            def p_mm(j, pool):
                jsl = slice(j * JW, (j + 1) * JW)
                pps = pool.tile([JW, H], F32, tag="pp", name="pp", bufs=2)
                for k in range(KA):
                    nc.tensor.matmul(
                        pps[:], attrk[k][:, jsl], w2k[k][:],
                        start=(k == 0), stop=(k == KA - 1),
                    )
                nc.vector.tensor_copy(Psb[j][:], pps[:])

            def consume_j(j):
                for ih in range(2):
                    isl = slice(ih * IW, (ih + 1) * IW)
                    nc.tensor.matmul(
                        csps[ih][:], ones_bc[:], expT[j][:, isl],
                        start=(j == 0), stop=(j == NJ - 1),
                    )
                    nc.tensor.matmul(
                        ep0[ih][:], Psb[j][:, 0:P], expT[j][:, isl],
                        start=(j == 0), stop=(j == NJ - 1),
                    )

            # j-loop: P/cs/EP-m0 interleaved so PE stays loaded past the
            # exp arrival rate and ramps to full clock; imgb at the tail
            with tc.tile_pool(name="psLoop", bufs=1, space="PSUM") as psL:
                e_chain(0)
                p_mm(0, psL)
                for j in range(NJ):
                    if j + 1 < NJ:
                        e_chain(j + 1)
                        p_mm(j + 1, psL)
                    consume_j(j)
            with tc.tile_pool(name="psImg", bufs=1, space="PSUM") as psI:
                imps = psI.tile([P, HM * BS], F32, tag="img", name="img")
                for m in range(HM):
                    for k in range(KA):
                        nc.tensor.matmul(
                            imps[:, m * BS:(m + 1) * BS],
                            imgw[:, k * H + m * P:k * H + (m + 1) * P],
                            imgfT[:, k * BS:(k + 1) * BS],
                            start=(k == 0), stop=(k == KA - 1),
                        )
                for m in range(HM):
                    nc.scalar.activation(
                        imgb[:, m * BS:(m + 1) * BS],
                        imps[:, m * BS:(m + 1) * BS],
                        AF.Identity, bias=sembT[:, m:m + 1],
                    )
            # reciprocal straight off the psum colsum broadcast, EP-m1
            # block, then sem2-m0/m1
            with tc.tile_pool(name="psEP1", bufs=1, space="PSUM") as psEP1:
                ep1 = [psEP1.tile([P, IW], F32, tag=f"ep1_{ih}",
                                  name=f"ep1_{ih}") for ih in range(2)]
                for ih in range(2):
                    isl = slice(ih * IW, (ih + 1) * IW)
                    nc.vector.reciprocal_approx_fast(
                        out=rb[:, isl], in_=csps[ih][:],
                    )
                msl = slice(P, 2 * P)
                for ih in range(2):
                    isl = slice(ih * IW, (ih + 1) * IW)
                    for j in range(NJ):
                        nc.tensor.matmul(
                            ep1[ih][:], Psb[j][:, msl], expT[j][:, isl],
                            start=(j == 0), stop=(j == NJ - 1),
                        )
                sem2_block(0, ep0)
                sem2_block(1, ep1)
        with tc.tile_pool(name="ps2", bufs=1, space="PSUM") as ps2:
            # four single-m waves with their own accumulators; EP-m2/m3
            # blocks slot between waves, sem2 right behind each
            p_m = [ps2.tile([JW, NJ * BS], F32, tag=f"pall{m}",
                            name=f"pall{m}") for m in range(HM)]
            relation_wave(0, p_m[0])
            ep2 = ep_block(ps2, 2, "ep2_")
            sem2_block(2, ep2)
            relation_wave(1, p_m[1])
            ep3 = ep_block(ps2, 3, "ep3_")
            sem2_block(3, ep3)
            relation_wave(2, p_m[2])
            pav = [p_m[m][:, :].rearrange("r (j b) -> r j b", j=NJ)
                   for m in range(HM)]
            osv = out_sb[:].rearrange("r (j b) -> r j b", j=NJ)
            dov = d_out.rearrange("(j r) b -> r j b", j=NJ)
            hb = BS // 2
            # final wave split in halves; out = sum of the four psum tiles
            for h0, h1 in ((0, hb), (hb, BS)):
                relation_wave(3, p_m[3], h0, h1)
                nc.scalar.copy(osv[:, :, h0:h1], pav[0][:, :, h0:h1])
                for m in range(1, HM):
                    nc.vector.tensor_tensor(
                        osv[:, :, h0:h1], osv[:, :, h0:h1],
                        pav[m][:, :, h0:h1], op=OP.add,
                    )
                nc.sync.dma_start(dov[:, :, h0:h1], osv[:, :, h0:h1])

    rpool_ctx.__exit__(None, None, None)
    epool_ctx.__exit__(None, None, None)
    cpool_ctx.__exit__(None, None, None)


def _prepare_in_maps(image_feats, attributes, att_w, att_a, img_w, sem_w,
                     sem_b, fc_w, fc_b):
    f = np.float32
    bf = ml_dtypes.bfloat16
    attributes = np.asarray(attributes, f)
    att_w = np.asarray(att_w, f)
    att_a = np.asarray(att_a, f)
    image_feats = np.asarray(image_feats, f)
    sem_w = np.asarray(sem_w, f)

    # attrT packed [128, (k, N)]
    attrT = np.ascontiguousarray(
        attributes.T.reshape(KA, P, N).transpose(1, 0, 2).reshape(P, KA * N)
    ).astype(bf)
    a1, a2 = att_a[:H, 0], att_a[H:, 0]
    w12f = np.stack([att_w @ a1, att_w @ a2], axis=1).astype(f)    # [A, 2]
    w12 = np.ascontiguousarray(
        w12f.reshape(KA, P, 2).transpose(1, 0, 2).reshape(P, 2 * KA)
    ).astype(bf)
    # w1 column replicated JW wide so f1b comes straight off one matmul
    w1rep = np.ascontiguousarray(
        np.repeat(w12f[:, 0:1], JW, axis=1)                        # [A, JW]
        .reshape(KA, P, JW).transpose(1, 0, 2).reshape(P, KA * JW)
    ).astype(bf)
    w2 = (att_w @ sem_w).astype(f)                                  # [A, H]
    w2 = np.ascontiguousarray(
        w2.reshape(KA, P, H).transpose(1, 0, 2).reshape(P, KA * H)
    ).astype(bf)

    def pack_k(w):
        return np.ascontiguousarray(
            np.asarray(w, f).reshape(KA, P, H).transpose(1, 0, 2)
            .reshape(P, KA * H)
        )

    img_w = pack_k(img_w).astype(bf)
    sem_bT = np.ascontiguousarray(np.asarray(sem_b, f).reshape(HM, P).T)
    fcw = np.ascontiguousarray(
        np.asarray(fc_w, f).reshape(HM, P).T
    ).astype(bf)

    shared = {
        "attrT": attrT, "w12": w12, "w1rep": w1rep,
        "w2": w2, "img_w": img_w, "sem_bT": sem_bT, "fcw": fcw,
    }
    in_maps = []
    for c in range(NCORES):
        imgfT = np.ascontiguousarray(
            image_feats[c * BS:(c + 1) * BS, :].T
            .reshape(KA, P, BS).transpose(1, 0, 2).reshape(P, KA * BS)
        ).astype(bf)
        in_maps.append(dict(shared, imgfT=imgfT))
    return in_maps


def _postprocess(outs, fc_b):
    """outs: list of per-core [N, BS] arrays -> full [B, N] + fc_b."""
    full = np.empty((B, N), np.float32)
    for c in range(NCORES):
        full[c * BS:(c + 1) * BS, :] = np.asarray(outs[c], np.float32).T
    return full + np.float32(np.asarray(fc_b).reshape(()))


def _make_runner(nc, in_maps):
    """Build the sharded PJRT callable once (mirrors
    bass2jax.run_bass_via_pjrt's multi-core path) so repeated kernel()
    calls reuse the compiled NEFF executable."""
    import jax
    from jax.sharding import Mesh, PartitionSpec

    try:
        from jax.experimental.shard_map import shard_map
    except ImportError:
        shard_map = jax.shard_map
    from concourse import bass2jax

    bass2jax.install_neuronx_cc_hook()
    n_cores = len(in_maps)
    partition_name = (
        nc.partition_id_tensor.name if nc.partition_id_tensor else None
    )
    in_names, out_names, out_avals = [], [], []
    for alloc in nc.m.functions[0].allocations:
        if not isinstance(alloc, mybir.MemoryLocationSet):
            continue
        name = alloc.memorylocations[0].name
        if alloc.kind == "ExternalInput":
            if name != partition_name:
                in_names.append(name)
        elif alloc.kind == "ExternalOutput":
            out_names.append(name)
            out_avals.append(
                jax.core.ShapedArray(
                    tuple(alloc.tensor_shape), mybir.dt.np(alloc.dtype)
                )
            )
    all_in_names = list(in_names) + list(out_names)
    if partition_name is not None:
        all_in_names.append(partition_name)
    n_params, n_outs = len(in_names), len(out_avals)

    def _body(*args):
        operands = list(args)
        if partition_name is not None:
            operands.append(bass2jax.partition_id_tensor())
        return tuple(bass2jax._bass_exec_p.bind(
            *operands,
            out_avals=tuple(out_avals),
            in_names=tuple(all_in_names),
            out_names=tuple(out_names),
            lowering_input_output_aliases=(),
            sim_require_finite=True,
            sim_require_nnan=True,
            nc=nc,
        ))

    donate = tuple(range(n_params, n_params + n_outs))
    devices = jax.devices()[:n_cores]
    mesh = Mesh(np.asarray(devices), ("core",))
    sharded = jax.jit(
        shard_map(
            _body, mesh=mesh,
            in_specs=(PartitionSpec("core"),) * (n_params + n_outs),
            out_specs=(PartitionSpec("core"),) * n_outs,
            check_rep=False,
        ),
        donate_argnums=donate, keep_unused=True,
    )

    import zlib

    def call(maps):
        concat_in = [
            np.concatenate([np.asarray(maps[c][n]) for c in range(n_cores)], 0)
            for n in in_names
        ]
        # keep inputs device-resident across calls with identical data
        key = tuple(zlib.adler32(x.tobytes()) for x in concat_in)
        dev = _CACHE.get("dev_inputs")
        if dev is None or dev[0] != key:
            dev = (key, [jax.device_put(x) for x in concat_in])
            _CACHE["dev_inputs"] = dev
        zeros = [
            np.zeros((n_cores * av.shape[0], *av.shape[1:]), av.dtype)
            for av in out_avals
        ]
        outs = sharded(*dev[1], *zeros)
        jax.block_until_ready(outs)
        oi = out_names.index("out")
        full = np.asarray(outs[oi]).reshape(n_cores, *out_avals[oi].shape)
        return list(full)

    return call


def run(inputs, **spmd_kwargs):
    """Returns (full output [B, N], BassKernelResults) via the generic
    run_bass_kernel_spmd path (used by test tooling)."""
    nc = _build_program()
    in_maps = _prepare_in_maps(**inputs)
    res = run_bass_kernel_spmd(nc, in_maps, list(range(NCORES)), **spmd_kwargs)
    out = _postprocess(
        [res.results[c]["out"] for c in range(NCORES)], inputs["fc_b"]
    )
    return out, res


def kernel(**inputs):
    nc = _build_program()
    in_maps = _prepare_in_maps(**inputs)
    if "runner" not in _CACHE:
        _CACHE["runner"] = _make_runner(nc, in_maps)
    return _postprocess(_CACHE["runner"](in_maps), inputs["fc_b"])


# revision 102
# speedup vs baseline: 1.0172x; 1.0172x over previous
"""Trainium2 Bass kernel for GATRelationNet (self-contained).

Math:
  att_h = attributes @ att_w                        [N, H]
  e     = leaky_relu(att_h@a1 + (att_h@a2).T, 0.2)  [N, N]
  attn  = softmax(e, axis=1)
  att_outs = attn @ att_h                           [N, H]
  img_proj = image_feats @ img_w                    [B, H]
  sem_proj = att_outs @ sem_w + sem_b               [N, H]
  out[b,n] = fc_b + sum_h fc_w[h]*relu(img_proj[b,h] + sem_proj[n,h])

Strategy (8 cores, GAT replicated, relation sharded over batch):
  - Weight folds on host: w12 = att_w @ [a1|a2]  (logit projections) and
    W2 = att_w @ sem_w, so sem_proj^T (unnormalized) is
    EP[h',i] = sum_j P[j,h'] * exp(e[i,j]) with P = attributes @ W2.
    att_h is never materialized on device.
  - bf16 operands throughout the GAT (f32 PSUM accumulation), which also
    enables the DVE 4x element mode for the big relu pass.
  - e-chain per j-chunk pipelines across engines (ACT fused Prelu for the
    first chunks, DVE z+leaky for the rest, ACT exp); P matmuls, the
    broadcast-form colsum, and EP-m0 partials interleave per chunk on PE
    so it stays continuously busy (full-clock pstate) behind the exps.
  - The colsum matmul uses a [JW,128] all-ones stationary so every psum
    row holds the column sum; the softmax reciprocal runs directly on
    that broadcast (one custom-DVE op), feeding sem2 = EP * (1/s).
  - Relation: relu tiles r[m,b] = relu(sem2T[m] + imgb[m][:,b]) [128,1000]
    produced whole-tile on DVE/ACT/GPSIMD (statically load balanced); the
    fc reduction uses r slices as the matmul *stationary* operand with the
    fc_w chunk moving -> out [125,1] per matmul (cost = 1 output column).
    Four single-m waves each accumulate into their own PSUM tile
    [125, 8j*32b] with one-shot groups (PSUM zero-region-safe); the
    output is the sum of the four, assembled in b-halves to pipeline the
    final copy + DMA.
  - Output leaves the device as out^T [N, BS]; the host transposes and
    adds fc_b.
"""

import numpy as np
import ml_dtypes

import concourse.bass as bass
import concourse.mybir as mybir
import concourse.tile as tile
from concourse import bacc
from concourse.bass_utils import run_bass_kernel_spmd

P = 128
B, N, A, H, IDIM = 256, 1000, 512, 512, 512
NCORES = 8
BS = B // NCORES      # 32 batch rows per core
KA = A // P           # 4 contraction chunks over A (and over H for EP)
HM = H // P           # 4 h chunks
NJ = 8                # j chunks
JW = N // NJ          # 125
IW = 500              # i half width (PSUM bank = 512 fp32)
NEG = 0.2

F32 = mybir.dt.float32
BF = mybir.dt.bfloat16
AF = mybir.ActivationFunctionType
OP = mybir.AluOpType

# relation relu producer split: engine per (m,b) tile index, built from
# rate weights (ns per [128,1000] tile) measured in TimelineSim.
RELU_DVE, RELU_ACT, RELU_GPS = 89, 24, 15   # sums to 128


def _relu_schedule():
    """Engine per relu tile (2 waves x 64 tiles), interleaved so each
    engine's queue drains evenly."""
    shares = {"D": RELU_DVE, "A": RELU_ACT, "G": RELU_GPS}
    acc = {k: 0.0 for k in shares}
    out = []
    for _ in range(128):
        for k in shares:
            acc[k] += shares[k] / 128.0
        pick = max(acc, key=lambda k: acc[k])
        acc[pick] -= 1.0
        out.append(pick)
    return out


_CACHE = {}


def _build_program():
    if "nc" in _CACHE:
        return _CACHE["nc"]

    nc = bacc.Bacc(
        "TRN2", target_bir_lowering=False, debug=False, num_devices=NCORES
    )

    d_attrT = nc.dram_tensor("attrT", [P, KA * N], BF, kind="ExternalInput")
    d_w12 = nc.dram_tensor("w12", [P, KA * 2], BF, kind="ExternalInput")
    d_w1rep = nc.dram_tensor("w1rep", [P, KA * JW], BF, kind="ExternalInput")
    d_w2 = nc.dram_tensor("w2", [P, KA * H], BF, kind="ExternalInput")
    d_img_w = nc.dram_tensor("img_w", [P, KA * H], BF, kind="ExternalInput")
    d_imgfT = nc.dram_tensor("imgfT", [P, KA * BS], BF, kind="ExternalInput")
    d_sem_bT = nc.dram_tensor("sem_bT", [P, HM], F32, kind="ExternalInput")
    d_fcw = nc.dram_tensor("fcw", [P, HM], BF, kind="ExternalInput")
    d_out = nc.dram_tensor("out", [N, BS], F32, kind="ExternalOutput")

    with tile.TileContext(nc) as tc:
        _program(nc, tc, d_attrT, d_w12, d_w1rep, d_w2, d_img_w,
                 d_imgfT, d_sem_bT, d_fcw, d_out)

    nc.compile()
    _CACHE["nc"] = nc
    return nc


def _program(nc, tc, d_attrT, d_w12, d_w1rep, d_w2, d_img_w,
             d_imgfT, d_sem_bT, d_fcw, d_out):
    cpool_ctx = tc.tile_pool(name="consts", bufs=1)
    cpool = cpool_ctx.__enter__()

    # ---- persistent SBUF tensors ----
    attrT = cpool.tile([P, KA * N], BF, tag="attrT", name="attrT")
    attrk = [attrT[:, k * N:(k + 1) * N] for k in range(KA)]
    w12 = cpool.tile([P, KA * 2], BF, tag="w12", name="w12")
    w1rep = cpool.tile([P, KA * JW], BF, tag="w1rep", name="w1rep")
    w2 = cpool.tile([P, KA * H], BF, tag="w2", name="w2")
    w2k = [w2[:, k * H:(k + 1) * H] for k in range(KA)]
    imgw = cpool.tile([P, KA * H], BF, tag="imgw", name="imgw")
    imgfT = cpool.tile([P, KA * BS], BF, tag="imgfT", name="imgfT")
    sembT = cpool.tile([P, HM], F32, tag="sembT", name="sembT")
    fcw = cpool.tile([P, HM], BF, tag="fcw", name="fcw")

    ones_row = cpool.tile([1, P], BF, tag="ones_row", name="ones_row")
    ones_bc = cpool.tile([JW, P], BF, tag="ones_bc", name="ones_bc")
    f1b = cpool.tile([JW, N], BF, tag="f1b", name="f1b")
    f2col = cpool.tile([JW, 2 * NJ], F32, tag="f2col", name="f2col")
    imgb = cpool.tile([P, HM * BS], F32, tag="imgb", name="imgb")
    expT = [cpool.tile([JW, N], BF, tag=f"expT{j}", name=f"expT{j}")
            for j in range(NJ)]
    Psb = [cpool.tile([JW, H], BF, tag=f"P{j}", name=f"P{j}")
           for j in range(NJ)]
    rb = cpool.tile([P, N], F32, tag="rb", name="rb")
    sem2T = [cpool.tile([P, N], BF, tag=f"s2{m}", name=f"s2{m}")
             for m in range(HM)]
    out_sb = cpool.tile([JW, NJ * BS], F32, tag="out_sb", name="out_sb")

    # ---- loads (ordered by first use; attrT chunked so the logit
    # matmuls start as each contraction chunk lands) ----
    nc.sync.dma_start(w1rep[:], d_w1rep[:, :])
    nc.sync.dma_start(w12[:], d_w12[:, :])
    for k in range(KA):
        ksl = slice(k * N, (k + 1) * N)
        nc.sync.dma_start(attrT[:, ksl], d_attrT[:, ksl])
    nc.sync.dma_start(w2[:], d_w2[:, :])
    nc.sync.dma_start(imgw[:], d_img_w[:, :])
    nc.sync.dma_start(imgfT[:], d_imgfT[:, :])
    nc.sync.dma_start(sembT[:], d_sem_bT[:, :])
    nc.sync.dma_start(fcw[:], d_fcw[:, :])

    nc.vector.memset(ones_row[:], 1.0)
    nc.vector.memset(ones_bc[:], 1.0)

    # warm the gpsimd ucode ops early (op load is ~us on real hw)
    gwarm = cpool.tile([P, 8], F32, tag="gwarm", name="gwarm")
    nc.vector.memset(gwarm[:], 0.0)
    nc.gpsimd.tensor_scalar(
        gwarm[:], gwarm[:], 0.0, 0.0, op0=OP.add, op1=OP.max
    )
    nc.gpsimd.tensor_tensor(
        gwarm[:], gwarm[:], gwarm[:], op=OP.mult
    )

    # ---- phase A: warmup, f1, f2, f1b, imgb (small PE matmuls) ----
    with tc.tile_pool(name="psA", bufs=1, space="PSUM") as psA:
        # PE pstate warmup: ~3us of continuous dummy matmuls while the
        # attrT DMA lands, so f1/f2/P run at full clock (0.42ns/col
        # instead of 0.83).
        wsrc = cpool.tile([1, IW], BF, tag="wsrc", name="wsrc")
        nc.vector.memset(wsrc[:], 0.0)
        for w in range(5):
            wps = psA.tile([P, IW], F32, tag="warm", name="warm", bufs=2)
            nc.tensor.matmul(wps[:], ones_row[0:1, :], wsrc[:],
                             start=True, stop=True)

        # f1b [125, N] produced directly by PE via host-replicated w1 columns
        for ih in range(2):
            isl = slice(ih * IW, (ih + 1) * IW)
            fps = psA.tile([JW, IW], F32, tag="f1b", name="f1b", bufs=2)
            for k in range(KA):
                nc.tensor.matmul(
                    fps[:], w1rep[:, k * JW:(k + 1) * JW], attrk[k][:, isl],
                    start=(k == 0), stop=(k == KA - 1),
                )
            nc.vector.tensor_copy(f1b[0:JW, isl], fps[:])

        f2ps = psA.tile([JW, 2 * NJ], F32, tag="f2", name="f2")
        for j in range(NJ):
            jsl = slice(j * JW, (j + 1) * JW)
            for k in range(KA):
                nc.tensor.matmul(
                    f2ps[:, 2 * j:2 * j + 2], attrk[k][:, jsl],
                    w12[:, 2 * k:2 * k + 2],
                    start=(k == 0), stop=(k == KA - 1),
                )
        nc.vector.tensor_copy(f2col[:], f2ps[:])

    # ---- phases B+C interleaved so PE stays continuously busy ----
    # j-loop: e-chain (DVE z -> DVE/GPS leaky -> ACT exp), P[j] matmuls,
    # EP-m0 + colsum partials.  Then EP-m1 / relation-m0 / EP-m2 /
    # relation-m1 / ... alternate: each EP-m block keeps PE busy while the
    # relu producers work, and each relation-m block drains fast.
    sched = _relu_schedule()
    epool_ctx = tc.tile_pool(name="et", bufs=3)
    epool = epool_ctx.__enter__()
    rpool_ctx = tc.tile_pool(name="relu", bufs=40)
    rpool = rpool_ctx.__enter__()

    def relu_tile(m, b, eng):
        r = rpool.tile([P, N], BF, tag="r", name="r")
        bias = imgb[:, m * BS + b:m * BS + b + 1]
        if eng == "A":
            nc.scalar.activation(r[:], sem2T[m][:], AF.Relu, bias=bias)
        elif eng == "D":
            nc.vector.tensor_scalar(
                r[:], sem2T[m][:], bias, 0.0, op0=OP.add, op1=OP.max
            )
        else:
            nc.gpsimd.tensor_scalar(
                r[:], sem2T[m][:], bias, 0.0, op0=OP.add, op1=OP.max
            )
        return r

    def relation_wave(m, p_acc, b0=0, b1=BS):
        # single-m wave: each PSUM column group is one start+stop matmul,
        # so only one accumulation group is ever open per zero region
        rs = {b: relu_tile(m, b, sched[m * BS + b]) for b in range(b0, b1)}
        for b in range(b0, b1):
            for j in range(NJ):
                jsl = slice(j * JW, (j + 1) * JW)
                nc.tensor.matmul(
                    p_acc[:, j * BS + b:j * BS + b + 1],
                    rs[b][:, jsl], fcw[:, m:m + 1],
                    start=True, stop=True,
                )

    def ep_block(pool, m, prefix):
        tiles = [pool.tile([P, IW], F32, tag=f"{prefix}{ih}",
                           name=f"{prefix}{ih}") for ih in range(2)]
        msl = slice(m * P, (m + 1) * P)
        for j in range(NJ):
            for ih in range(2):
                isl = slice(ih * IW, (ih + 1) * IW)
                nc.tensor.matmul(
                    tiles[ih][:], Psb[j][:, msl], expT[j][:, isl],
                    start=(j == 0), stop=(j == NJ - 1),
                )
        return tiles

    def sem2_block(m, tiles):
        # GPSIMD cannot touch PSUM: m0 multiplies on DVE (gate-critical);
        # the rest stage through SBUF via ACT and multiply on GPS
        for ih in range(2):
            isl = slice(ih * IW, (ih + 1) * IW)
            if m < 2:
                nc.vector.tensor_tensor(
                    sem2T[m][:, isl], tiles[ih][:], rb[:, isl], op=OP.mult,
                )
            else:
                scr = epool.tile([P, IW], BF, tag="s2scr", name="s2scr")
                nc.scalar.copy(scr[:], tiles[ih][:])
                nc.gpsimd.tensor_tensor(
                    sem2T[m][:, isl], scr[:], rb[:, isl], op=OP.mult,
                )

    def e_chain(j):
        # first chunks: fused Prelu on ACT (fills ACT while the DVE/GPS
        # leakys for later chunks run ahead); exp always on ACT, which is
        # the pipeline pacer.
        e_t = epool.tile([JW, N], BF, tag="e", name="e")
        if j < 4:
            nc.scalar.activation(
                e_t[:], f1b[:], AF.Prelu,
                bias=f2col[:, 2 * j + 1:2 * j + 2], alpha=NEG,
            )
        else:
            nc.vector.tensor_scalar(
                e_t[:], f1b[:], f2col[:, 2 * j + 1:2 * j + 2],
                None, op0=OP.add,
            )
            nc.vector.scalar_tensor_tensor(
                e_t[:], e_t[:], NEG, e_t[:], op0=OP.mult, op1=OP.max
            )
        nc.scalar.activation(expT[j][:], e_t[:], AF.Exp)

    if True:
        with tc.tile_pool(name="ps1", bufs=1, space="PSUM") as ps1:
            # colsum accumulated directly in broadcast form: lhsT is a full
            # [JW, P] ones block, so every psum row holds the column sum and
            # the reciprocal can run straight on it (no copy/bcast hops)
            csps = [ps1.tile([P, IW], F32, tag=f"cs{ih}", name=f"cs{ih}")
                    for ih in range(2)]
            ep0 = [ps1.tile([P, IW], F32, tag=f"ep0_{ih}",
                            name=f"ep0_{ih}") for ih in range(2)]

            def p_mm(j, pool):
                jsl = slice(j * JW, (j + 1) * JW)
                pps = pool.tile([JW, H], F32, tag="pp", name="pp", bufs=2)
                for k in range(KA):
                    nc.tensor.matmul(
                        pps[:], attrk[k][:, jsl], w2k[k][:],
                        start=(k == 0), stop=(k == KA - 1),
                    )
                nc.vector.tensor_copy(Psb[j][:], pps[:])

            def consume_j(j):
                for ih in range(2):
                    isl = slice(ih * IW, (ih + 1) * IW)
                    nc.tensor.matmul(
                        csps[ih][:], ones_bc[:], expT[j][:, isl],
                        start=(j == 0), stop=(j == NJ - 1),
                    )
                    nc.tensor.matmul(
                        ep0[ih][:], Psb[j][:, 0:P], expT[j][:, isl],
                        start=(j == 0), stop=(j == NJ - 1),
                    )

            # j-loop: P/cs/EP-m0 interleaved so PE stays loaded past the
            # exp arrival rate and ramps to full clock; imgb at the tail
            with tc.tile_pool(name="psLoop", bufs=1, space="PSUM") as psL:
                e_chain(0)
                p_mm(0, psL)
                for j in range(NJ):
                    if j + 1 < NJ:
                        e_chain(j + 1)
                        p_mm(j + 1, psL)
                    consume_j(j)
            with tc.tile_pool(name="psImg", bufs=1, space="PSUM") as psI:
                imps = psI.tile([P, HM * BS], F32, tag="img", name="img")
                for m in range(HM):
                    for k in range(KA):
                        nc.tensor.matmul(
                            imps[:, m * BS:(m + 1) * BS],
                            imgw[:, k * H + m * P:k * H + (m + 1) * P],
                            imgfT[:, k * BS:(k + 1) * BS],
                            start=(k == 0), stop=(k == KA - 1),
                        )
                for m in range(HM):
                    nc.scalar.activation(
                        imgb[:, m * BS:(m + 1) * BS],
                        imps[:, m * BS:(m + 1) * BS],
                        AF.Identity, bias=sembT[:, m:m + 1],
                    )
            # reciprocal straight off the psum colsum broadcast, EP-m1
            # block, then sem2-m0/m1
            with tc.tile_pool(name="psEP1", bufs=1, space="PSUM") as psEP1:
                ep1 = [psEP1.tile([P, IW], F32, tag=f"ep1_{ih}",
                                  name=f"ep1_{ih}") for ih in range(2)]
                for ih in range(2):
                    isl = slice(ih * IW, (ih + 1) * IW)
                    nc.vector.reciprocal_approx_fast(
                        out=rb[:, isl], in_=csps[ih][:],
                    )
                msl = slice(P, 2 * P)
                for ih in range(2):
                    isl = slice(ih * IW, (ih + 1) * IW)
                    for j in range(NJ):
                        nc.tensor.matmul(
                            ep1[ih][:], Psb[j][:, msl], expT[j][:, isl],
                            start=(j == 0), stop=(j == NJ - 1),
                        )
                sem2_block(0, ep0)
                sem2_block(1, ep1)
        with tc.tile_pool(name="ps2", bufs=1, space="PSUM") as ps2:
            # four single-m waves with their own accumulators; EP-m2/m3
            # blocks slot between waves, sem2 right behind each
            p_m = [ps2.tile([JW, NJ * BS], F32, tag=f"pall{m}",
                            name=f"pall{m}") for m in range(HM)]
            relation_wave(0, p_m[0])
            ep2 = ep_block(ps2, 2, "ep2_")
            sem2_block(2, ep2)
            relation_wave(1, p_m[1])
            ep3 = ep_block(ps2, 3, "ep3_")
            sem2_block(3, ep3)
            relation_wave(2, p_m[2])
            pav = [p_m[m][:, :].rearrange("r (j b) -> r j b", j=NJ)
                   for m in range(HM)]
            osv = out_sb[:].rearrange("r (j b) -> r j b", j=NJ)
            dov = d_out.rearrange("(j r) b -> r j b", j=NJ)
            hb = BS // 2
            # final wave split in halves; out = sum of the four psum tiles
            for h0, h1 in ((0, hb), (hb, BS)):
                relation_wave(3, p_m[3], h0, h1)
                nc.scalar.copy(osv[:, :, h0:h1], pav[0][:, :, h0:h1])
                for m in range(1, HM):
                    nc.vector.tensor_tensor(
                        osv[:, :, h0:h1], osv[:, :, h0:h1],
                        pav[m][:, :, h0:h1], op=OP.add,
                    )
                nc.sync.dma_start(dov[:, :, h0:h1], osv[:, :, h0:h1])

    rpool_ctx.__exit__(None, None, None)
    epool_ctx.__exit__(None, None, None)
    cpool_ctx.__exit__(None, None, None)


def _prepare_in_maps(image_feats, attributes, att_w, att_a, img_w, sem_w,
                     sem_b, fc_w, fc_b):
    f = np.float32
    bf = ml_dtypes.bfloat16
    attributes = np.asarray(attributes, f)
    att_w = np.asarray(att_w, f)
    att_a = np.asarray(att_a, f)
    image_feats = np.asarray(image_feats, f)
    sem_w = np.asarray(sem_w, f)

    # attrT packed [128, (k, N)]
    attrT = np.ascontiguousarray(
        attributes.T.reshape(KA, P, N).transpose(1, 0, 2).reshape(P, KA * N)
    ).astype(bf)
    a1, a2 = att_a[:H, 0], att_a[H:, 0]
    w12f = np.stack([att_w @ a1, att_w @ a2], axis=1).astype(f)    # [A, 2]
    w12 = np.ascontiguousarray(
        w12f.reshape(KA, P, 2).transpose(1, 0, 2).reshape(P, 2 * KA)
    ).astype(bf)
    # w1 column replicated JW wide so f1b comes straight off one matmul
    w1rep = np.ascontiguousarray(
        np.repeat(w12f[:, 0:1], JW, axis=1)                        # [A, JW]
        .reshape(KA, P, JW).transpose(1, 0, 2).reshape(P, KA * JW)
    ).astype(bf)
    w2 = (att_w @ sem_w).astype(f)                                  # [A, H]
    w2 = np.ascontiguousarray(
        w2.reshape(KA, P, H).transpose(1, 0, 2).reshape(P, KA * H)
    ).astype(bf)

    def pack_k(w):
        return np.ascontiguousarray(
            np.asarray(w, f).reshape(KA, P, H).transpose(1, 0, 2)
            .reshape(P, KA * H)
        )

    img_w = pack_k(img_w).astype(bf)
    sem_bT = np.ascontiguousarray(np.asarray(sem_b, f).reshape(HM, P).T)
    fcw = np.ascontiguousarray(
        np.asarray(fc_w, f).reshape(HM, P).T
    ).astype(bf)

    shared = {
        "attrT": attrT, "w12": w12, "w1rep": w1rep,
        "w2": w2, "img_w": img_w, "sem_bT": sem_bT, "fcw": fcw,
    }
    in_maps = []
    for c in range(NCORES):
        imgfT = np.ascontiguousarray(
            image_feats[c * BS:(c + 1) * BS, :].T
            .reshape(KA, P, BS).transpose(1, 0, 2).reshape(P, KA * BS)
        ).astype(bf)
        in_maps.append(dict(shared, imgfT=imgfT))
    return in_maps


def _postprocess(outs, fc_b):
    """outs: list of per-core [N, BS] arrays -> full [B, N] + fc_b."""
    full = np.empty((B, N), np.float32)
    for c in range(NCORES):
        full[c * BS:(c + 1) * BS, :] = np.asarray(outs[c], np.float32).T
    return full + np.float32(np.asarray(fc_b).reshape(()))


def _make_runner(nc, in_maps):
    """Build the sharded PJRT callable once (mirrors
    bass2jax.run_bass_via_pjrt's multi-core path) so repeated kernel()
    calls reuse the compiled NEFF executable."""
    import jax
    from jax.sharding import Mesh, PartitionSpec

    try:
        from jax.experimental.shard_map import shard_map
    except ImportError:
        shard_map = jax.shard_map
    from concourse import bass2jax

    bass2jax.install_neuronx_cc_hook()
    n_cores = len(in_maps)
    partition_name = (
        nc.partition_id_tensor.name if nc.partition_id_tensor else None
    )
    in_names, out_names, out_avals = [], [], []
    for alloc in nc.m.functions[0].allocations:
        if not isinstance(alloc, mybir.MemoryLocationSet):
            continue
        name = alloc.memorylocations[0].name
        if alloc.kind == "ExternalInput":
            if name != partition_name:
                in_names.append(name)
        elif alloc.kind == "ExternalOutput":
            out_names.append(name)
            out_avals.append(
                jax.core.ShapedArray(
                    tuple(alloc.tensor_shape), mybir.dt.np(alloc.dtype)
                )
            )
    all_in_names = list(in_names) + list(out_names)
    if partition_name is not None:
        all_in_names.append(partition_name)
    n_params, n_outs = len(in_names), len(out_avals)

    def _body(*args):
        operands = list(args)
        if partition_name is not None:
            operands.append(bass2jax.partition_id_tensor())
        return tuple(bass2jax._bass_exec_p.bind(
            *operands,
            out_avals=tuple(out_avals),
            in_names=tuple(all_in_names),
            out_names=tuple(out_names),
            lowering_input_output_aliases=(),
            sim_require_finite=True,
            sim_require_nnan=True,
            nc=nc,
        ))

    donate = tuple(range(n_params, n_params + n_outs))
    devices = jax.devices()[:n_cores]
    mesh = Mesh(np.asarray(devices), ("core",))
    sharded = jax.jit(
        shard_map(
            _body, mesh=mesh,
            in_specs=(PartitionSpec("core"),) * (n_params + n_outs),
            out_specs=(PartitionSpec("core"),) * n_outs,
            check_rep=False,
        ),
        donate_argnums=donate, keep_unused=True,
    )

    import zlib

    def call(maps):
        concat_in = [
            np.concatenate([np.asarray(maps[c][n]) for c in range(n_cores)], 0)
            for n in in_names
        ]
        # keep inputs device-resident across calls with identical data
        key = tuple(zlib.adler32(x.tobytes()) for x in concat_in)
        dev = _CACHE.get("dev_inputs")
        if dev is None or dev[0] != key:
            dev = (key, [jax.device_put(x) for x in concat_in])
            _CACHE["dev_inputs"] = dev
        zeros = [
            np.zeros((n_cores * av.shape[0], *av.shape[1:]), av.dtype)
            for av in out_avals
        ]
        outs = sharded(*dev[1], *zeros)
        jax.block_until_ready(outs)
        oi = out_names.index("out")
        full = np.asarray(outs[oi]).reshape(n_cores, *out_avals[oi].shape)
        return list(full)

    return call


def run(inputs, **spmd_kwargs):
    """Returns (full output [B, N], BassKernelResults) via the generic
    run_bass_kernel_spmd path (used by test tooling)."""
    nc = _build_program()
    in_maps = _prepare_in_maps(**inputs)
    res = run_bass_kernel_spmd(nc, in_maps, list(range(NCORES)), **spmd_kwargs)
    out = _postprocess(
        [res.results[c]["out"] for c in range(NCORES)], inputs["fc_b"]
    )
    return out, res


def kernel(**inputs):
    nc = _build_program()
    in_maps = _prepare_in_maps(**inputs)
    if "runner" not in _CACHE:
        _CACHE["runner"] = _make_runner(nc, in_maps)
    return _postprocess(_CACHE["runner"](in_maps), inputs["fc_b"])
